# revision 29
# baseline (speedup 1.0000x reference)
"""Trainium2 Bass kernel for nn_BagInput (segment_reduce).

Pipeline per core (data-parallel over contiguous segment ranges):
  h   = LeakyReLU(concat(feats, mask, ones) @ W_aug.T)        (PE + ACT)
  agg = segment_sum(h) / len                                   (PE matmul with 0/1 selection)
  out = LayerNorm(agg) * gamma + beta                          (PE transpose + DVE/ACT)

All matmul operands fp16 (fp32 PSUM accumulate); LayerNorm in fp32.
"""
import sys
import os

sys.path.insert(0, "/opt/trn_rl_repo")

import numpy as np
import orjson

import concourse.bass as bass
import concourse.tile as tile
from concourse import mybir
from concourse.bass_utils import run_bass_kernel_spmd

FEAT = 64
NMASK = 16
FDIM = FEAT + NMASK + 1  # 81: feats + mask + ones column (bias)
BAG = 128
LEAK = 0.01
LN_EPS = 1e-5
NCORES = 8
TILE = 128            # items per tile (partition dim)
GROUP = 8             # tiles per group (one leaky / xt-copy batch)
SUPER = 8             # groups per input-DMA super chunk
SEGBLK = 512          # segments per psum block
F16 = mybir.dt.float16
F32 = mybir.dt.float32


# ---------------------------------------------------------------------------
# BIR post-pass: this container's neuronxcc walrus accepts only ONE sync-wait
# per instruction; Tile attaches several.  Waiting on monotonic semaphores
# one-at-a-time in program order on the same engine is equivalent.
def _split_multi_waits(bir_bytes: bytes) -> bytes:
    mod = orjson.loads(bir_bytes)
    n = 0
    for fn in mod["functions"]:
        for bb in fn["blocks"]:
            out = []
            for ins in bb["instructions"]:
                si = ins.get("sync_info")
                waits = si.get("on_wait") if si else None
                if waits and len(waits) > 1:
                    for w in waits[:-1]:
                        n += 1
                        nop = {
                            "engine": ins["engine"],
                            "ins": [],
                            "name": f"WSPLIT-{n}",
                            "opcode": "NoOp",
                            "outs": [],
                            "sync_info": {"on_update": [], "on_wait": [w]},
                        }
                        if "debug" in ins:
                            nop["debug"] = ins["debug"]
                        out.append(nop)
                    si["on_wait"] = [waits[-1]]
                out.append(ins)
            bb["instructions"] = out
    return orjson.dumps(mod)


def _patch_bass(nc):
    orig = nc.to_json_bytes
    nc.to_json_bytes = lambda: _split_multi_waits(orig())
    return nc


# ---------------------------------------------------------------------------
# Host-side structure: per-core segment pieces for the segment-sum matmuls.
def _build_structure(x_len_core: np.ndarray, i_pad: int):
    """Pieces: [tile, block, psum_off, width, a_off, start, final] per
    128-item tile; windows cover every column of every 512-seg block."""
    x_len_core = x_len_core.astype(np.int64)
    n_items = int(x_len_core.sum())
    n_seg = len(x_len_core)
    nblk = (n_seg + SEGBLK - 1) // SEGBLK
    seg_of_item = np.repeat(np.arange(n_seg), x_len_core)
    t_pad = i_pad // TILE

    raw = []  # [tile, block, lo, hi) in block-local cols
    for t in range(t_pad):
        lo_i, hi_i = t * TILE, min((t + 1) * TILE, n_items)
        if lo_i >= n_items:
            break
        s0, s1 = int(seg_of_item[lo_i]), int(seg_of_item[hi_i - 1])
        for b in range(s0 // SEGBLK, s1 // SEGBLK + 1):
            sa, sb = max(s0, b * SEGBLK), min(s1, b * SEGBLK + SEGBLK - 1)
            raw.append([t, b, sa - b * SEGBLK, sb - b * SEGBLK + 1])

    # coverage: first piece of a block starts at 0, gaps filled by extending
    # the next piece down, last piece of a block extends to SEGBLK.
    pieces = []
    for b in range(nblk):
        plist = [p for p in raw if p[1] == b]
        assert plist, f"block {b} has no items"
        prev_end = 0
        for k, (t, _b, lo, hi) in enumerate(plist):
            lo = min(lo, prev_end)
            if k == 0:
                lo = 0
            if k == len(plist) - 1:
                hi = SEGBLK
            pieces.append([t, b, lo, hi - lo, 0, int(k == 0), int(k == len(plist) - 1)])
            prev_end = hi

    # A-blob columns
    a_off = 0
    for p in pieces:
        p[4] = a_off
        a_off += p[3]
    w_total = a_off

    # A entries are 1/len(seg): the matmul then produces segment MEANS
    # directly.  fp16 rounding of 1/len scales a whole segment row uniformly,
    # which cancels exactly in LayerNorm.
    recip = (1.0 / np.maximum(x_len_core, 1)).astype(np.float32)
    a_blob = np.zeros((TILE, w_total), dtype=np.float16)
    for t, b, lo, w, aoff, _st, _fin in pieces:
        lo_i, hi_i = t * TILE, min((t + 1) * TILE, n_items)
        segs = seg_of_item[lo_i:hi_i]
        rel = segs - (b * SEGBLK + lo)
        rows = np.arange(hi_i - lo_i)
        m = (rel >= 0) & (rel < w)
        a_blob[rows[m], aoff + rel[m]] = recip[segs[m]]
    return pieces, a_blob, nblk


def _build_kernel(t_pad, nblk, pieces, chunk_cols, w_total, s_pad, apply_gb):
    """Build the Bass/Tile kernel. Structure must be identical across cores."""
    n_groups = t_pad // GROUP
    i_pad = t_pad * TILE
    nc = bass.Bass()

    feats_in = nc.dram_tensor("feats", [i_pad, FEAT], F32, kind="ExternalInput")
    mask_in = nc.dram_tensor("mask", [i_pad, NMASK], F32, kind="ExternalInput")
    wt_in = nc.dram_tensor("wt", [FDIM, BAG], F16, kind="ExternalInput")
    id16_in = nc.dram_tensor("id16", [128, 128], F16, kind="ExternalInput")
    id32_in = nc.dram_tensor("id32", [128, 128], F32, kind="ExternalInput")
    a_in = nc.dram_tensor("ablob", [TILE, w_total], F16, kind="ExternalInput")
    if apply_gb:
        gb_in = nc.dram_tensor("gammab", [128, 2, BAG], F32, kind="ExternalInput")
    out_t = nc.dram_tensor("out", [s_pad, BAG], F32, kind="ExternalOutput")

    # group pieces by tile for the emit loop
    pieces_by_tile = {}
    for p in pieces:
        pieces_by_tile.setdefault(p[0], []).append(p)

    # views
    out_v = out_t[:].rearrange("(b q p) f -> b p q f", p=128, q=4)
    n_super = (n_groups + SUPER - 1) // SUPER

    def chunk_view(t, s, j_s):
        lo = s * GROUP * SUPER * TILE
        return t[lo : lo + j_s * TILE, :].rearrange("(j p) f -> p j f", p=TILE)

    with tile.TileContext(nc) as tc:
        with (
            tc.tile_pool(name="const", bufs=1) as const,
            tc.tile_pool(name="xp", bufs=3) as xp,
            tc.tile_pool(name="xtp", bufs=4) as xtp,
            tc.tile_pool(name="hp", bufs=4) as hp,
            tc.tile_pool(name="ap", bufs=3) as apool,
            tc.tile_pool(name="aggp", bufs=3) as aggp,
            tc.tile_pool(name="lnp", bufs=8) as lnp,
            tc.tile_pool(name="outp", bufs=4) as outp,
            tc.tile_pool(name="ps_tp", bufs=2, space="PSUM") as ps_tp,
            tc.tile_pool(name="ps_h", bufs=2, space="PSUM") as ps_h,
            tc.tile_pool(name="ps_seg", bufs=2, space="PSUM") as ps_seg,
        ):
            wt_sb = const.tile([FDIM, BAG], F16, tag="wt")
            nc.sync.dma_start(wt_sb, wt_in[:])
            id16 = const.tile([128, 128], F16, tag="id16")
            nc.sync.dma_start(id16, id16_in[:])
            id32 = const.tile([128, 128], F32, tag="id32")
            nc.sync.dma_start(id32, id32_in[:])
            eps_sb = const.tile([128, 1], F32, tag="eps")
            nc.vector.memset(eps_sb, LN_EPS)
            if apply_gb:
                gb_sb = const.tile([128, 2, BAG], F32, tag="gb")
                nc.sync.dma_start(gb_sb, gb_in[:])

            seg_tiles = {}   # block -> psum tile
            x_cur = None
            a_cur = None
            a_base = 0

            for g in range(n_groups):
                s = g // SUPER
                if g % SUPER == 0:
                    # super-chunk input DMA (SWDGE casts f32 -> f16)
                    j_s = min(GROUP * SUPER, (n_groups - s * SUPER) * GROUP)
                    x_cur = xp.tile([TILE, j_s, FDIM], F16, tag="x", name=f"x{s}")
                    fv = chunk_view(feats_in[:], s, j_s)
                    mvw = chunk_view(mask_in[:], s, j_s)
                    # first chunk: per-group slices so the pipeline primes fast;
                    # steady state: halves (finer dep granularity than whole).
                    cuts = (
                        list(range(0, j_s, 2 * GROUP)) + [j_s]
                        if s == 0
                        else [0, (j_s + 1) // 2, j_s]
                    )
                    for ja, jb in zip(cuts[:-1], cuts[1:]):
                        if jb <= ja:
                            continue
                        nc.gpsimd.dma_start(
                            out=x_cur[:, ja:jb, 0:FEAT], in_=fv[:, ja:jb, :]
                        )
                        nc.gpsimd.dma_start(
                            out=x_cur[:, ja:jb, FEAT : FEAT + NMASK],
                            in_=mvw[:, ja:jb, :],
                        )
                    nc.vector.memset(x_cur[:, :, FDIM - 1 : FDIM], 1.0)
                    # A-blob chunk for this super chunk
                    lo_c, hi_c = chunk_cols[s]
                    if hi_c > lo_c:
                        a_cur = apool.tile([TILE, hi_c - lo_c], F16, tag="a")
                        nc.sync.dma_start(a_cur, a_in[:, lo_c:hi_c])
                        a_base = lo_c

                jg = (g % SUPER) * GROUP  # tile offset inside super chunk

                # transposes: x [128it, 81] -> xt [81, 128it]
                xt_ps = ps_tp.tile([FDIM, GROUP * 128], F16, tag="tp")
                for j in range(GROUP):
                    nc.tensor.transpose(
                        xt_ps[:, j * 128 : (j + 1) * 128], x_cur[:, jg + j, :], id16
                    )
                xt_sb = xtp.tile([FDIM, GROUP * 128], F16, tag="xt")
                nc.vector.tensor_copy(xt_sb, xt_ps)

                # mm1: h[it, bag] = xt.T @ wt
                h_ps = ps_h.tile([128, GROUP * 128], F32, tag="h")
                for j in range(GROUP):
                    nc.tensor.matmul(
                        h_ps[:, j * 128 : (j + 1) * 128],
                        xt_sb[:, j * 128 : (j + 1) * 128],
                        wt_sb,
                        start=True,
                        stop=True,
                    )
                h_sb = hp.tile([128, GROUP * 128], F16, tag="hs")
                nc.scalar.activation(
                    out=h_sb, in_=h_ps, func=mybir.ActivationFunctionType.Lrelu,
                    bias=0.0, scale=1.0, alpha=LEAK,
                )

                # mm2 segment-sum pieces for the GROUP tiles of this group
                for j in range(GROUP):
                    t = g * GROUP + j
                    for (tt, b, lo, w, aoff, st, fin) in pieces_by_tile.get(t, []):
                        if b not in seg_tiles:
                            seg_tiles[b] = ps_seg.tile(
                                [128, SEGBLK], F32, tag="seg", name=f"seg{b}"
                            )
                        nc.tensor.matmul(
                            seg_tiles[b][:, lo : lo + w],
                            h_sb[:, j * 128 : (j + 1) * 128],
                            a_cur[:, aoff - a_base : aoff - a_base + w],
                            start=bool(st),
                            stop=bool(fin),
                            skip_group_check=True,
                        )
                        if fin:
                            _finalize_block(
                                nc, b, seg_tiles.pop(b), aggp, lnp, outp,
                                id32, eps_sb,
                                gb_sb if apply_gb else None,
                                ps_seg, out_v,
                            )
    return _patch_bass(nc)


def _finalize_block(nc, b, seg_ps, aggp, lnp, outp, id32, eps_sb, gb_sb,
                    ps_seg, out_v):
    """seg_ps [128bag, 512seg] fp32 means -> transpose -> LN -> DMA."""
    agg = aggp.tile([128, SEGBLK], F32, tag="agg")
    nc.vector.tensor_copy(agg, seg_ps)
    # reuse the seg psum pool: block b's slot frees after the copy above
    t_ps = ps_seg.tile([128, SEGBLK], F32, tag="seg", name=f"tps{b}")
    for q in range(4):
        nc.tensor.transpose(
            t_ps[:, q * 128 : (q + 1) * 128], agg[:, q * 128 : (q + 1) * 128], id32
        )
    agg2 = aggp.tile([128, SEGBLK], F32, tag="agg2")
    nc.vector.tensor_copy(agg2, t_ps)
    out_sb = outp.tile([128, 4, BAG], F32, tag="out")
    for q in range(4):
        aq = agg2[:, q * 128 : (q + 1) * 128]  # [128seg, 128bag] = seg means
        stats = lnp.tile([128, 6], F32, tag="stats")
        nc.vector.bn_stats(stats, aq)
        mv = lnp.tile([128, 2], F32, tag="mv")
        nc.vector.bn_aggr(mv, stats)
        nc.scalar.activation(
            out=mv[:, 1:2], in_=mv[:, 1:2],
            func=mybir.ActivationFunctionType.Sqrt,
            bias=eps_sb[:, 0:1], scale=1.0,
        )
        nc.vector.reciprocal(mv[:, 1:2], mv[:, 1:2])
        nc.vector.tensor_scalar(
            out=out_sb[:, q, :], in0=aq,
            scalar1=mv[:, 0:1], scalar2=mv[:, 1:2],
            op0=mybir.AluOpType.subtract, op1=mybir.AluOpType.mult,
        )
        if gb_sb is not None:
            nc.vector.tensor_mul(out_sb[:, q, :], out_sb[:, q, :], gb_sb[:, 0, :])
            nc.vector.tensor_add(out_sb[:, q, :], out_sb[:, q, :], gb_sb[:, 1, :])
    nc.sync.dma_start(out_v[b], out_sb)


# ---------------------------------------------------------------------------
def kernel(feats, mask, W, b, gamma, beta, x_len):
    feats = np.asarray(feats, dtype=np.float32)
    mask = np.asarray(mask, dtype=np.float32)
    W = np.asarray(W, dtype=np.float32)
    b = np.asarray(b, dtype=np.float32)
    gamma = np.asarray(gamma, dtype=np.float32)
    beta = np.asarray(beta, dtype=np.float32)
    x_len = np.asarray(x_len, dtype=np.int32)

    n_seg = len(x_len)
    ends = np.cumsum(x_len, dtype=np.int64)

    # shard: equal contiguous segment ranges per core
    seg_bounds = [round(c * n_seg / NCORES) for c in range(NCORES + 1)]
    item_bounds = [0] + [int(ends[sb - 1]) if sb > 0 else 0 for sb in seg_bounds[1:]]

    core_lens = [x_len[seg_bounds[c] : seg_bounds[c + 1]] for c in range(NCORES)]
    core_items = [item_bounds[c + 1] - item_bounds[c] for c in range(NCORES)]

    tile_group = TILE * GROUP
    i_pad = max(
        (max(core_items) + tile_group - 1) // tile_group * tile_group, tile_group
    )

    structs = [_build_structure(cl, i_pad) for cl in core_lens]
    shapes_equal = all(
        structs[c][2] == structs[0][2]
        and len(structs[c][0]) == len(structs[0][0])
        and np.array_equal(np.array(structs[c][0]), np.array(structs[0][0]))
        for c in range(NCORES)
    )
    item_ranges = [(item_bounds[c], item_bounds[c + 1]) for c in range(NCORES)]
    replicated = not shapes_equal
    if replicated:
        # fallback: replicate the full problem on every core (slow, correct)
        n_items = int(ends[-1]) if n_seg else 0
        core_lens = [x_len] * NCORES
        item_ranges = [(0, n_items)] * NCORES
        i_pad = max(
            (n_items + tile_group - 1) // tile_group * tile_group, tile_group
        )
        st = _build_structure(x_len, i_pad)
        structs = [st] * NCORES

    pieces0, _, nblk = structs[0]
    t_pad = i_pad // TILE
    n_groups = t_pad // GROUP
    n_super = (n_groups + SUPER - 1) // SUPER
    s_pad = nblk * SEGBLK
    w_total = pieces0[-1][4] + pieces0[-1][3]

    # A chunk boundaries per super-chunk: cols of pieces whose tile is inside
    chunk_cols = []
    for s in range(n_super):
        t_lo, t_hi = s * GROUP * SUPER, (s + 1) * GROUP * SUPER
        cols = [
            (p[4], p[4] + p[3]) for p in pieces0 if t_lo <= p[0] < t_hi
        ]
        if cols:
            chunk_cols.append((cols[0][0], cols[-1][1]))
        else:
            chunk_cols.append((0, 0))

    apply_gb = not (np.all(gamma == 1.0) and np.all(beta == 0.0))

    # host-prepped shared inputs
    wt_aug = np.concatenate(
        [W.T, b[None, :]], axis=0
    ).astype(np.float16)  # [81, 128]
    id16 = np.eye(128, dtype=np.float16)
    id32 = np.eye(128, dtype=np.float32)

    in_maps = []
    for c in range(NCORES):
        pieces, a_blob, _ = structs[c]
        i0, i1 = item_ranges[c]
        fpad = np.zeros((i_pad, FEAT), dtype=np.float32)
        fpad[: i1 - i0] = feats[i0:i1]
        mpad = np.zeros((i_pad, NMASK), dtype=np.float32)
        mpad[: i1 - i0] = mask[i0:i1]
        im = {
            "feats": fpad,
            "mask": mpad,
            "wt": wt_aug,
            "id16": id16,
            "id32": id32,
            "ablob": a_blob,
        }
        if apply_gb:
            im["gammab"] = np.stack(
                [np.tile(gamma[None, :], (128, 1)), np.tile(beta[None, :], (128, 1))],
                axis=1,
            ).astype(np.float32)
        in_maps.append(im)

    nc = _build_kernel(t_pad, nblk, pieces0, chunk_cols, w_total, s_pad, apply_gb)
    res = run_bass_kernel_spmd(nc, in_maps, core_ids=list(range(NCORES)))

    out = np.empty((n_seg, BAG), dtype=np.float32)
    if replicated:
        out[:] = res.results[0]["out"][:n_seg]
    else:
        for c in range(NCORES):
            out[seg_bounds[c] : seg_bounds[c + 1]] = res.results[c]["out"][
                : seg_bounds[c + 1] - seg_bounds[c]
            ]
    return out
